# revision 13
# baseline (speedup 1.0000x reference)
"""GATv2 3-layer GNN on 8 Trainium2 NeuronCores (Bass/Tile).

Sharding: nodes partitioned across 8 cores (12500 each); edges sharded by
destination node so segment-softmax and scatter-add stay device-local.

Per layer each core computes x_l/x_r for its own nodes (bf16 tables),
AllGathers x_l, then processes its edges in groups of G destination blocks
(128 dst nodes per block):
  - per-edge x_l[src] / x_r[dst] rows fetched with the GPSIMD dma_gather
    custom instruction (int16 indices, so the global x_l table is split in
    windows of 25000 rows; edge slots are ordered window-major inside each
    group and padded to whole 128-edge tiles per (block, window))
  - leaky_relu / attention logits / exp / message weighting on DVE+ACT slabs
  - segment softmax-sum + weighted scatter-add via a one-hot indicator
    matmul accumulated in PSUM (segment-max subtraction is skipped: logits
    for this model are in [-1.3, 1.3] so exp cannot overflow and the softmax
    is mathematically identical).
h activations are kept in DRAM and streamed block-wise.
"""

import math
from contextlib import ExitStack

import numpy as np

import concourse.bass as bass
import concourse.bacc as bacc
import concourse.mybir as mybir
import concourse.tile as tile
from concourse._compat import with_exitstack
from concourse.masks import make_identity

F32 = mybir.dt.float32
BF16 = mybir.dt.bfloat16
I16 = mybir.dt.int16
AX = mybir.AxisListType
OP = mybir.AluOpType
ACTF = mybir.ActivationFunctionType

P = 128
NEG_SLOPE = 0.2
EPS = 1e-5
WSZ = 25000          # int16 index window size for the global x_l table
G = 4                # dst blocks per edge-phase group


# ----------------------------------------------------------------------------
# static layout shared by host packer and kernel builder
# ----------------------------------------------------------------------------
class Layout:
    """Static slot/tile layout derived from T_bw [NBLK, W]."""

    def __init__(self, nblk, W, T_bw):
        self.nblk = nblk
        self.W = W
        self.T_bw = T_bw
        self.groups = [list(range(s, min(s + G, nblk))) for s in range(0, nblk, G)]
        # tile enumeration: for g, for w, for b in group, tiles
        self.tile_off = {}           # (g, w, b) -> first tile index
        self.chunk_tiles = {}        # (g, w) -> tiles in chunk
        self.chunk_off = {}          # (g, w) -> first tile index of chunk
        n = 0
        for g, blks in enumerate(self.groups):
            for w in range(W):
                self.chunk_off[(g, w)] = n
                for b in blks:
                    self.tile_off[(g, w, b)] = n
                    n += T_bw[b][w]
                self.chunk_tiles[(g, w)] = n - self.chunk_off[(g, w)]
        self.total_tiles = n
        self.slots = n * P


def compute_layout(edge_index, n_nodes, n_cores):
    N = n_nodes
    loops = np.arange(N, dtype=np.int64)
    src = np.concatenate([edge_index[0].astype(np.int64), loops])
    dst = np.concatenate([edge_index[1].astype(np.int64), loops])
    per = N // n_cores
    nblk = math.ceil(per / P)
    W = math.ceil(N / WSZ)

    core = dst // per
    dstl = dst - core * per
    blk = dstl // P
    win = src // WSZ
    key = (core * nblk + blk) * W + win
    cnt = np.bincount(key, minlength=n_cores * nblk * W).reshape(n_cores, nblk, W)
    T_bw = [[int(x) for x in row] for row in np.ceil(cnt.max(axis=0) / P).astype(np.int64)]
    lay = Layout(nblk, W, T_bw)
    ed = dict(src=src, dst=dst, core=core, dstl=dstl, blk=blk, win=win,
              cnt=cnt, per=per)
    return lay, ed


def prep_arrays(lay: Layout, ed, n_cores):
    """Per-core gather index / dst arrays in the static slot layout."""
    src, core = ed["src"], ed["core"]
    dstl, blk, win = ed["dstl"], ed["blk"], ed["win"]
    nblk, W = lay.nblk, lay.W
    nE = len(src)

    key = (core * nblk + blk) * W + win
    order = np.argsort(key, kind="stable")
    rank = np.empty(nE, dtype=np.int64)
    starts = np.zeros(n_cores * nblk * W + 1, dtype=np.int64)
    np.cumsum(ed["cnt"].reshape(-1), out=starts[1:])
    rank[order] = np.arange(nE) - starts[key[order]]

    t = rank // P
    p = rank % P
    g = blk // G
    tile_off_arr = np.zeros((len(lay.groups), W, nblk), dtype=np.int64)
    for (gg, ww, bb), off in lay.tile_off.items():
        tile_off_arr[gg, ww, bb] = off
    tile_idx = tile_off_arr[g, win, blk] + t
    slot = tile_idx * P + p

    S = lay.slots
    TI = lay.total_tiles
    idx_xl = np.zeros((n_cores, S), dtype=np.int16)
    idx_xr = np.zeros((n_cores, S), dtype=np.int16)
    dstc = np.full((n_cores, P, TI), -1.0, dtype=np.float32)
    idx_xl[core, slot] = (src - win * WSZ).astype(np.int16)
    idx_xr[core, slot] = dstl.astype(np.int16)
    dstc[core, p, tile_idx] = (dstl - blk * P).astype(np.float32)

    # wrap indices per call (g, w): call-relative pos q -> [q % 16, q // 16]
    def wrap(a):
        out = np.zeros((n_cores, 16, S // 16), dtype=np.int16)
        for (g_, w_), ct in lay.chunk_tiles.items():
            base = lay.chunk_off[(g_, w_)] * P
            n = ct * P
            if n == 0:
                continue
            seg = a[:, base:base + n].reshape(n_cores, n // 16, 16)
            out[:, :, base // 16:(base + n) // 16] = seg.transpose(0, 2, 1)
        return out

    return dict(idx_xl=wrap(idx_xl), idx_xr=wrap(idx_xr), dstc=dstc)


def make_att_tile(att):
    flat = np.ascontiguousarray(att, dtype=np.float32).reshape(-1)
    assert flat.shape[0] == 128
    return np.tile(flat[None, :], (P, 1)).astype(np.float32).copy()


# ----------------------------------------------------------------------------
# config
# ----------------------------------------------------------------------------
class Cfg:
    def __init__(self, n_nodes, f_in, hid, heads, n_cores, lay: Layout):
        assert n_nodes % n_cores == 0
        self.N = n_nodes
        self.F_IN = f_in
        self.HID = hid
        self.HEADS = heads
        self.CORES = n_cores
        self.PER = n_nodes // n_cores
        self.NBLK = math.ceil(self.PER / P)
        self.LASTB = self.PER - (self.NBLK - 1) * P
        self.lay = lay


# ----------------------------------------------------------------------------
# kernel builder
# ----------------------------------------------------------------------------
def build_kernel(cfg: Cfg):
    nc = bacc.Bacc("TRN2", num_devices=cfg.CORES, debug=False,
                   target_bir_lowering=False)
    lay = cfg.lay
    N, PER, NBLK = cfg.N, cfg.PER, cfg.NBLK
    HID, F_IN, HEADS = cfg.HID, cfg.F_IN, cfg.HEADS
    W = lay.W
    S16 = lay.slots // 16
    TI = lay.total_tiles
    groups = [list(range(cfg.CORES))]

    # ---------------- kernel I/O ----------------
    x_in = nc.dram_tensor("x_loc", [PER, F_IN], F32, kind="ExternalInput")
    ixl_in = nc.dram_tensor("idx_xl", [16, S16], I16, kind="ExternalInput")
    ixr_in = nc.dram_tensor("idx_xr", [16, S16], I16, kind="ExternalInput")
    dstc_in = nc.dram_tensor("dstc", [P, TI], F32, kind="ExternalInput")
    iota_in = nc.dram_tensor("iota_row", [P, P], F32, kind="ExternalInput")

    w_in = {}
    for l, fin in ((1, F_IN), (2, HID), (3, HID)):
        w_in[f"wl{l}"] = nc.dram_tensor(f"wl{l}", [fin, HID], F32, kind="ExternalInput")
        w_in[f"wr{l}"] = nc.dram_tensor(f"wr{l}", [fin, HID], F32, kind="ExternalInput")
        w_in[f"att{l}"] = nc.dram_tensor(f"att{l}", [P, P], F32, kind="ExternalInput")
        w_in[f"g{l}"] = nc.dram_tensor(f"g{l}", [1, HID], F32, kind="ExternalInput")
        w_in[f"b{l}"] = nc.dram_tensor(f"b{l}", [1, HID], F32, kind="ExternalInput")
    gin_in = nc.dram_tensor("g_in", [1, F_IN], F32, kind="ExternalInput")
    bin_in = nc.dram_tensor("b_in", [1, F_IN], F32, kind="ExternalInput")
    for hd in ("rtt", "ret"):
        w_in[f"{hd}_w1"] = nc.dram_tensor(f"{hd}_w1", [HID, HID // 2], F32, kind="ExternalInput")
        w_in[f"{hd}_b1"] = nc.dram_tensor(f"{hd}_b1", [HID // 2, 1], F32, kind="ExternalInput")
        w_in[f"{hd}_w2"] = nc.dram_tensor(f"{hd}_w2", [HID // 2, 1], F32, kind="ExternalInput")
        w_in[f"{hd}_b2"] = nc.dram_tensor(f"{hd}_b2", [1, 1], F32, kind="ExternalInput")
    out_t = nc.dram_tensor("out", [PER, 2], F32, kind="ExternalOutput")

    # internal DRAM
    h_dram = [nc.dram_tensor(f"hd{i}", [PER, HID], F32, kind="Internal")
              for i in range(2)]
    h0_dram = nc.dram_tensor("h0d", [PER, F_IN], F32, kind="Internal")
    xl_loc, xr_loc, xl_full = [], [], []
    for l in range(3):
        xl_loc.append(nc.dram_tensor(f"xl_loc{l}", [PER, HID], BF16, kind="Internal"))
        xr_loc.append(nc.dram_tensor(f"xr_loc{l}", [PER, HID], BF16, kind="Internal"))
        # NOTE: not addr_space="Shared" — the dma_gather Q7 ucode cannot
        # address the Shared scratchpad window (device crash); Local output
        # AllGather is supported, just slightly slower.
        xl_full.append(nc.dram_tensor(f"xl_full{l}", [N, HID], BF16,
                                      kind="Internal"))

    MAXGT = max(sum(lay.chunk_tiles[(g, w)] for w in range(W))
                for g in range(len(lay.groups)))
    MAXCT = max(lay.chunk_tiles.values())

    @with_exitstack
    def kern(ctx: ExitStack, tc: tile.TileContext):
        persist = ctx.enter_context(tc.tile_pool(name="persist", bufs=1))
        gsl = ctx.enter_context(tc.tile_pool(name="gsl", bufs=2))      # group slabs
        wsl = ctx.enter_context(tc.tile_pool(name="wsl", bufs=W + 1))  # chunk slabs
        sb = ctx.enter_context(tc.tile_pool(name="sb", bufs=2))
        sbs = ctx.enter_context(tc.tile_pool(name="sbs", bufs=3))
        pp = ctx.enter_context(tc.tile_pool(name="pp", bufs=6, space="PSUM"))

        # ---------------- persistent SBUF ----------------
        ixl_sb = persist.tile([P, S16], I16)
        nc.sync.dma_start(ixl_sb[:], bass.AP(ixl_in, 0, [[0, 8], [S16, 16], [1, S16]]))
        ixr_sb = persist.tile([P, S16], I16)
        nc.sync.dma_start(ixr_sb[:], bass.AP(ixr_in, 0, [[0, 8], [S16, 16], [1, S16]]))
        dstc_sb = persist.tile([P, TI], F32)
        nc.sync.dma_start(dstc_sb[:], dstc_in[:])
        iota = persist.tile([P, P], F32)
        nc.sync.dma_start(iota[:], iota_in[:])
        ident = persist.tile([P, P], F32)
        make_identity(nc, ident[:])
        eps_sb = persist.tile([P, 1], F32)
        nc.vector.memset(eps_sb[:], EPS)

        wsb = {}
        for l, fin in ((1, F_IN), (2, HID), (3, HID)):
            for nm in (f"wl{l}", f"wr{l}"):
                wsb[nm] = persist.tile([fin, HID], F32, name=nm)
                nc.sync.dma_start(wsb[nm][:], w_in[nm][:])
            nm = f"att{l}"
            wsb[nm] = persist.tile([P, P], BF16, name=f"{nm}_sb")
            nc.gpsimd.dma_start(wsb[nm][:], w_in[nm][:])   # f32 -> bf16 cast
            for nm in (f"g{l}", f"b{l}"):
                wsb[nm] = persist.tile([P, HID], F32, name=f"{nm}_sb")
                nc.sync.dma_start(wsb[nm][:], w_in[nm][:].to_broadcast([P, HID]))
        g_in_sb = persist.tile([P, F_IN], F32)
        nc.sync.dma_start(g_in_sb[:], gin_in[:].to_broadcast([P, F_IN]))
        b_in_sb = persist.tile([P, F_IN], F32)
        nc.sync.dma_start(b_in_sb[:], bin_in[:].to_broadcast([P, F_IN]))
        for hd in ("rtt", "ret"):
            for nm, shp in ((f"{hd}_w1", [HID, HID // 2]), (f"{hd}_b1", [HID // 2, 1]),
                            (f"{hd}_w2", [HID // 2, 1])):
                wsb[nm] = persist.tile(shp, F32, name=f"{nm}_sb")
                nc.sync.dma_start(wsb[nm][:], w_in[nm][:])
            nm = f"{hd}_b2"
            wsb[nm] = persist.tile([P, 1], F32, name=f"{nm}_sb")
            nc.sync.dma_start(wsb[nm][:], w_in[nm][:].to_broadcast([P, 1]))

        def na_of(b):
            return cfg.LASTB if b == NBLK - 1 else P

        # -------- layernorm helper --------
        def layer_norm(src_ap, dest_ap, na, F, g_sb, b_sb, post):
            sm = sbs.tile([P, 1], F32, tag="ln_sm")
            nc.vector.reduce_sum(out=sm[:na], in_=src_ap, axis=AX.X)
            mn = sbs.tile([P, 1], F32, tag="ln_mn")
            nc.scalar.mul(mn[:na], sm[:na], 1.0 / F)
            sq = sb.tile([P, F], F32, tag="ln_sq")
            nc.scalar.activation(out=sq[:na], in_=src_ap, func=ACTF.Square)
            s2 = sbs.tile([P, 1], F32, tag="ln_s2")
            nc.vector.reduce_sum(out=s2[:na], in_=sq[:na], axis=AX.X)
            var = sbs.tile([P, 1], F32, tag="ln_var")
            nc.vector.tensor_scalar(out=var[:na], in0=s2[:na], scalar1=1.0 / F,
                                    scalar2=None, op0=OP.mult)
            m2 = sbs.tile([P, 1], F32, tag="ln_m2")
            nc.vector.tensor_tensor(out=m2[:na], in0=mn[:na], in1=mn[:na], op=OP.mult)
            nc.vector.tensor_tensor(out=var[:na], in0=var[:na], in1=m2[:na],
                                    op=OP.subtract)
            std = sbs.tile([P, 1], F32, tag="ln_std")
            nc.scalar.activation(out=std[:na], in_=var[:na], func=ACTF.Sqrt,
                                 bias=eps_sb[:na])
            rstd = sbs.tile([P, 1], F32, tag="ln_rstd")
            nc.vector.reciprocal(out=rstd[:na], in_=std[:na])
            xn = sb.tile([P, F], F32, tag="ln_xn")
            nc.vector.tensor_scalar(out=xn[:na], in0=src_ap, scalar1=mn[:na],
                                    scalar2=rstd[:na], op0=OP.subtract, op1=OP.mult)
            nc.vector.tensor_tensor(out=xn[:na], in0=xn[:na], in1=g_sb[:na], op=OP.mult)
            if post == "relu":
                nc.vector.tensor_tensor(out=xn[:na], in0=xn[:na], in1=b_sb[:na],
                                        op=OP.add)
                nc.scalar.activation(out=dest_ap, in_=xn[:na], func=ACTF.Relu)
            elif post == "none":
                nc.vector.tensor_tensor(out=dest_ap, in0=xn[:na], in1=b_sb[:na],
                                        op=OP.add)
            else:
                _, res_ap, relu = post
                nc.vector.tensor_tensor(out=xn[:na], in0=xn[:na], in1=b_sb[:na],
                                        op=OP.add)
                r01 = sb.tile([P, F], F32, tag="ln_r01")
                nc.scalar.mul(r01[:na], res_ap, 0.1)
                if relu:
                    nc.vector.tensor_tensor(out=xn[:na], in0=xn[:na], in1=r01[:na],
                                            op=OP.add)
                    nc.scalar.activation(out=dest_ap, in_=xn[:na], func=ACTF.Relu)
                else:
                    nc.vector.tensor_tensor(out=dest_ap, in0=xn[:na], in1=r01[:na],
                                            op=OP.add)

        # -------- phase 0: ln_in(x) -> h0_dram --------
        for b in range(NBLK):
            na = na_of(b)
            xb = sb.tile([P, F_IN], F32, tag="x_blk")
            nc.sync.dma_start(xb[:na], x_in[b * P: b * P + na, :])
            hob = sb.tile([P, F_IN], F32, tag="h0_blk")
            layer_norm(xb[:na], hob[:na], na, F_IN, g_in_sb, b_in_sb, "none")
            nc.sync.dma_start(h0_dram[b * P: b * P + na, :], hob[:na])

        # -------- x-side: h -> xl_loc, xr_loc (bf16 DRAM) --------
        def x_side(l, h_src, F):
            wl = wsb[f"wl{l + 1}"]
            wr = wsb[f"wr{l + 1}"]
            for b in range(NBLK):
                na = na_of(b)
                hb = sb.tile([P, F], F32, tag="xs_h")
                if na < P:
                    nc.vector.memset(hb[:], 0.0)
                nc.sync.dma_start(hb[:na], h_src[b * P: b * P + na, :])
                pt = pp.tile([P, P], F32, tag="pp", space="PSUM")
                nc.tensor.transpose(out=pt[:F, :], in_=hb[:, :F], identity=ident[:])
                hT = sb.tile([P, P], F32, tag="xs_hT")
                nc.scalar.copy(hT[:F, :], pt[:F, :])
                for w_sb, dram in ((wl, xl_loc[l]), (wr, xr_loc[l])):
                    pm = pp.tile([P, HID], F32, tag="pp", space="PSUM")
                    nc.tensor.matmul(out=pm[:], lhsT=hT[:F, :], rhs=w_sb[:],
                                     start=True, stop=True)
                    xs = sb.tile([P, HID], BF16, tag="xs_out")
                    nc.scalar.copy(xs[:na], pm[:na])
                    nc.sync.dma_start(dram[b * P: b * P + na, :], xs[:na])

        # -------- edge phase --------
        def edge_phase(l, h_dst_dram, h_res_dram):
            H = 1 if l == 2 else HEADS
            C = HID // H
            WID = HID + H
            att = wsb[f"att{l + 1}"]
            g_sb = wsb[f"g{l + 1}"]
            b_sb = wsb[f"b{l + 1}"]
            for g, blks in enumerate(lay.groups):
                g0 = lay.chunk_off[(g, 0)]      # first tile of group
                # --- gathers (one dma_gather per window chunk) ---
                xj = gsl.tile([P, MAXGT * HID], BF16, tag="e_xj")
                xr = gsl.tile([P, MAXGT * HID], BF16, tag="e_xr")
                for w in range(W):
                    ct = lay.chunk_tiles[(g, w)]
                    if ct == 0:
                        continue
                    coff = lay.chunk_off[(g, w)] - g0
                    n_i = ct * P
                    base = lay.chunk_off[(g, w)] * P
                    wlo = w * WSZ
                    whi = min(N, wlo + WSZ)
                    out_xj = bass.AP(xj.tensor, xj[:].offset + coff * HID,
                                     [list(xj[:].ap[0]), [HID, ct], [1, HID]])
                    nc.gpsimd.dma_gather(
                        out_ap=out_xj,
                        in_ap=xl_full[l][wlo:whi, :],
                        idxs_ap=ixl_sb[:, base // 16:(base + n_i) // 16],
                        num_idxs=n_i, num_idxs_reg=n_i, elem_size=HID,
                        single_packet=False)
                    out_xr = bass.AP(xr.tensor, xr[:].offset + coff * HID,
                                     [list(xr[:].ap[0]), [HID, ct], [1, HID]])
                    nc.gpsimd.dma_gather(
                        out_ap=out_xr,
                        in_ap=xr_loc[l][:],
                        idxs_ap=ixr_sb[:, base // 16:(base + n_i) // 16],
                        num_idxs=n_i, num_idxs_reg=n_i, elem_size=HID,
                        single_packet=False)

                # --- per window chunk: Sel + edge math ---
                sels = {}
                augs = {}
                for w in range(W):
                    ct = lay.chunk_tiles[(g, w)]
                    if ct == 0:
                        continue
                    coff = lay.chunk_off[(g, w)] - g0
                    a0 = coff * HID
                    xj_c = xj[:, a0:a0 + ct * HID]
                    xr_c = xr[:, a0:a0 + ct * HID]

                    sel = wsl.tile([P, MAXCT * P], BF16, tag="e_sel")
                    tcol = lay.chunk_off[(g, w)]
                    dstc_b = bass.AP(dstc_sb.tensor, dstc_sb[:, tcol:tcol + ct].offset,
                                     [list(dstc_sb[:].ap[0]), [1, ct], [0, P]])
                    iota_b = bass.AP(iota.tensor, iota[:].offset,
                                     [list(iota[:].ap[0]), [0, ct], [1, P]])
                    nc.vector.tensor_tensor(out=sel[:, :ct * P], in0=dstc_b,
                                            in1=iota_b, op=OP.is_equal)
                    sels[w] = sel

                    # s = xj + xr (into xr slab)
                    nc.vector.tensor_tensor(out=xr_c, in0=xr_c, in1=xj_c, op=OP.add)
                    # e = max(s, 0.2 s)
                    e02 = wsl.tile([P, MAXCT * HID], BF16, tag="e_e02")
                    e02_c = e02[:, :ct * HID]
                    nc.scalar.mul(e02_c, xr_c, NEG_SLOPE)
                    nc.vector.tensor_tensor(out=e02_c, in0=xr_c, in1=e02_c, op=OP.max)
                    # e * att
                    att_b = bass.AP(att.tensor, att[:].offset,
                                    [list(att[:].ap[0]), [0, ct], [1, HID]])
                    nc.vector.tensor_tensor(out=e02_c, in0=e02_c, in1=att_b,
                                            op=OP.mult)
                    # logits (fp32)
                    lg = sbs.tile([P, MAXCT * 8], F32, tag="e_lg")
                    e3d = bass.AP(e02.tensor, e02[:].offset,
                                  [list(e02[:].ap[0]), [C, ct * H], [1, C]])
                    nc.vector.reduce_sum(out=lg[:, :ct * H], in_=e3d, axis=AX.X)
                    # p = exp(logits) into aug cols [HID:HID+H) per tile
                    aug = wsl.tile([P, MAXCT * (HID + HEADS)], BF16, tag="e_aug")
                    p_out = bass.AP(aug.tensor, aug[:].offset + HID,
                                    [list(aug[:].ap[0]), [WID, ct], [1, H]])
                    nc.scalar.activation(out=p_out, in_=lg[:, :ct * H], func=ACTF.Exp)
                    # M' = xj * p
                    m_out = bass.AP(aug.tensor, aug[:].offset,
                                    [list(aug[:].ap[0]), [WID, ct], [C, H], [1, C]])
                    xj_b = bass.AP(xj.tensor, xj[:].offset + a0,
                                   [list(xj[:].ap[0]), [HID, ct], [C, H], [1, C]])
                    p_b = bass.AP(aug.tensor, aug[:].offset + HID,
                                  [list(aug[:].ap[0]), [WID, ct], [1, H], [0, C]])
                    nc.vector.tensor_tensor(out=m_out, in0=xj_b, in1=p_b, op=OP.mult)
                    augs[w] = aug

                # --- per block: scatter matmuls + normalize + LN ---
                for b in blks:
                    na = na_of(b)
                    pblk = pp.tile([P, WID], F32, tag="pp", space="PSUM")
                    mm = []
                    for w in range(W):
                        for t in range(lay.T_bw[b][w]):
                            pos = lay.tile_off[(g, w, b)] - lay.chunk_off[(g, w)] + t
                            mm.append((w, pos))
                    for i, (w, pos) in enumerate(mm):
                        nc.tensor.matmul(
                            out=pblk[:],
                            lhsT=sels[w][:, pos * P:(pos + 1) * P],
                            rhs=augs[w][:, pos * WID:(pos + 1) * WID],
                            start=(i == 0), stop=(i == len(mm) - 1))

                    rs = sbs.tile([P, 8], F32, tag="e_rs")
                    nc.vector.reciprocal(out=rs[:na, :H], in_=pblk[:na, HID:HID + H])
                    agg = sb.tile([P, HID], F32, tag="e_agg")
                    for h in range(H):
                        nc.vector.tensor_scalar(
                            out=agg[:na, h * C:(h + 1) * C],
                            in0=pblk[:na, h * C:(h + 1) * C],
                            scalar1=rs[:na, h:h + 1], scalar2=None, op0=OP.mult)

                    hout = sb.tile([P, HID], F32, tag="e_hout")
                    if l == 0:
                        post = "relu"
                    else:
                        res = sb.tile([P, HID], F32, tag="e_res")
                        nc.sync.dma_start(res[:na], h_res_dram[b * P: b * P + na, :])
                        post = ("res", res[:na], l == 1)
                    layer_norm(agg[:na], hout[:na], na, HID, g_sb, b_sb, post)
                    nc.sync.dma_start(h_dst_dram[b * P: b * P + na, :], hout[:na])

        # -------- layers --------
        # layer 0: h0 -> h_dram[0]; layer 1: h_dram[0] -> h_dram[1] (res h0's
        # successor h_dram[0]); layer 2: h_dram[1] -> h_dram[0] overwrite? keep
        # separate: use h_dram[0], h_dram[1] alternately; residual source is the
        # layer's input h.
        x_side(0, h0_dram, F_IN)
        nc.gpsimd.collective_compute(
            "AllGather", OP.bypass, replica_groups=groups,
            ins=[xl_loc[0][:]], outs=[xl_full[0][:]])
        edge_phase(0, h_dram[0], None)

        x_side(1, h_dram[0], HID)
        nc.gpsimd.collective_compute(
            "AllGather", OP.bypass, replica_groups=groups,
            ins=[xl_loc[1][:]], outs=[xl_full[1][:]])
        edge_phase(1, h_dram[1], h_dram[0])

        x_side(2, h_dram[1], HID)
        nc.gpsimd.collective_compute(
            "AllGather", OP.bypass, replica_groups=groups,
            ins=[xl_loc[2][:]], outs=[xl_full[2][:]])
        edge_phase(2, h_dram[0], h_dram[1])
        h3 = h_dram[0]

        # -------- prediction heads --------
        for b in range(NBLK):
            na = na_of(b)
            hb = sb.tile([P, HID], F32, tag="hd_h")
            if na < P:
                nc.vector.memset(hb[:], 0.0)
            nc.sync.dma_start(hb[:na], h3[b * P: b * P + na, :])
            pt = pp.tile([P, P], F32, tag="pp", space="PSUM")
            nc.tensor.transpose(out=pt[:], in_=hb[:], identity=ident[:])
            hT = sb.tile([P, P], F32, tag="hd_hT")
            nc.scalar.copy(hT[:], pt[:])
            ob = sbs.tile([P, 2], F32, tag="hd_out")
            for ci, hd in enumerate(("rtt", "ret")):
                pm1 = pp.tile([HID // 2, P], F32, tag="pp", space="PSUM")
                nc.tensor.matmul(out=pm1[:], lhsT=wsb[f"{hd}_w1"][:], rhs=hT[:],
                                 start=True, stop=True)
                a1 = sb.tile([HID // 2, P], F32, tag="hd_a1")
                nc.scalar.activation(out=a1[:], in_=pm1[:], func=ACTF.Relu,
                                     bias=wsb[f"{hd}_b1"][:])
                pm2 = pp.tile([P, 1], F32, tag="pp", space="PSUM")
                nc.tensor.matmul(out=pm2[:], lhsT=a1[:], rhs=wsb[f"{hd}_w2"][:],
                                 start=True, stop=True)
                nc.scalar.activation(out=ob[:na, ci:ci + 1], in_=pm2[:na],
                                     func=ACTF.Identity, bias=wsb[f"{hd}_b2"][:na])
            nc.sync.dma_start(out_t[b * P: b * P + na, :], ob[:na])

    with tile.TileContext(nc) as tc:
        kern(tc)
    nc.compile()
    return nc


# ----------------------------------------------------------------------------
# input packing
# ----------------------------------------------------------------------------
def make_in_maps(cfg: Cfg, inputs, prep):
    iota_row = np.tile(np.arange(P, dtype=np.float32)[None, :], (P, 1)).copy()
    in_maps = []
    for c in range(cfg.CORES):
        m = dict(
            x_loc=np.ascontiguousarray(
                inputs["x"][c * cfg.PER:(c + 1) * cfg.PER]).astype(np.float32),
            idx_xl=np.ascontiguousarray(prep["idx_xl"][c]),
            idx_xr=np.ascontiguousarray(prep["idx_xr"][c]),
            dstc=np.ascontiguousarray(prep["dstc"][c]),
            iota_row=iota_row,
            g_in=inputs["ln_in_g"].reshape(1, -1).astype(np.float32),
            b_in=inputs["ln_in_b"].reshape(1, -1).astype(np.float32),
        )
        for l in (1, 2, 3):
            m[f"wl{l}"] = np.ascontiguousarray(inputs[f"w_l{l}"]).astype(np.float32)
            m[f"wr{l}"] = np.ascontiguousarray(inputs[f"w_r{l}"]).astype(np.float32)
            m[f"att{l}"] = make_att_tile(inputs[f"att{l}"])
            m[f"g{l}"] = inputs[f"ln{l}_g"].reshape(1, -1).astype(np.float32)
            m[f"b{l}"] = inputs[f"ln{l}_b"].reshape(1, -1).astype(np.float32)
        for hd in ("rtt", "ret"):
            m[f"{hd}_w1"] = np.ascontiguousarray(inputs[f"{hd}_w1"]).astype(np.float32)
            m[f"{hd}_b1"] = inputs[f"{hd}_b1"].reshape(-1, 1).astype(np.float32)
            m[f"{hd}_w2"] = np.ascontiguousarray(inputs[f"{hd}_w2"]).astype(np.float32)
            m[f"{hd}_b2"] = inputs[f"{hd}_b2"].reshape(1, 1).astype(np.float32)
        in_maps.append(m)
    return in_maps


# ----------------------------------------------------------------------------
# public entry point
# ----------------------------------------------------------------------------
def run(inputs, n_cores=8, trace=False):
    inputs = {k: np.asarray(v) for k, v in inputs.items()}
    N, F_IN = inputs["x"].shape
    HID = inputs["w_l1"].shape[1]
    HEADS = inputs["att1"].shape[0]
    lay, ed = compute_layout(inputs["edge_index"], N, n_cores)
    prep = prep_arrays(lay, ed, n_cores)
    cfg = Cfg(N, F_IN, HID, HEADS, n_cores, lay)
    nc = build_kernel(cfg)
    in_maps = make_in_maps(cfg, inputs, prep)

    from concourse.bass_utils import run_bass_kernel_spmd
    res = run_bass_kernel_spmd(nc, in_maps, core_ids=list(range(n_cores)),
                               trace=trace)
    out = np.concatenate([res.results[c]["out"] for c in range(n_cores)], axis=0)
    return out, res


def kernel(**inputs):
    out, _ = run(inputs, n_cores=8, trace=False)
    return out.astype(np.float32)
